# revision 16
# baseline (speedup 1.0000x reference)
"""Trainium2 Bass kernel for nn_BTSPMemory: z = ((x_bits @ S.T) - mu) / std' / T.

Strategy (per sharding hint): shard x_bits along batch across the 8 cores,
replicate S / z_mu / z_std. Each core computes a [1024, 1000] slice of the
output with an fp8 DoubleRow matmul (contraction K = 16384) followed by an
affine epilogue (scale/bias per class) on the vector engine.

Host-side prep: bool -> fp8(0.0/1.0) bytes, transpose to K-major, tile so
every DMA is a straight per-partition-contiguous copy.
"""

import os
import sys

for _p in ("/opt/trn_rl_repo", "/root/.axon_site/_ro/trn_rl_repo"):
    if os.path.isdir(_p) and _p not in sys.path:
        sys.path.insert(0, _p)

from contextlib import ExitStack

import ml_dtypes
import numpy as np

import concourse.bacc as bacc
import concourse.bass as bass
import concourse.mybir as mybir
import concourse.tile as tile
from concourse.bass import ts
from concourse.bass_utils import run_bass_kernel_spmd

P = 128
FP8 = mybir.dt.float8e4
F32 = mybir.dt.float32
FP8_NP = ml_dtypes.float8_e4m3
ONE_FP8 = 0x38  # bit pattern of 1.0 in e4m3

# Problem shapes (hardcoded per contract)
B_FULL = 8192
C = 1000
K = 16384
N_CORES = 8
B_SHARD = B_FULL // N_CORES  # 1024
C_PAD = 1024
TEMPERATURE = 1.5

NT = C_PAD // 512  # moving tiles per k-step (2)


def build_nc(b_shard=B_SHARD, k=K, c=C, c_pad=C_PAD, s_chunk=16, passes=1,
             loop=False):
    """Build the per-core Bass program.

    DRAM inputs (per core):
      x  [b_shard, KS, 128] fp8 : x[mt*128 + p, ks, j] = x_bits[b=mt*128+j, n=ks*128+p]
      s  [128, KS, c_pad]   fp8 : s[p, ks, cc] = S[cc, ks*128+p] (zero-padded classes)
      za [128, c_pad] f32       : scale  (replicated over partitions)
      zb [128, c_pad] f32       : bias   (replicated over partitions)
    Output:
      out [b_shard, c] f32
    """
    KS = k // P  # 128-row k-subtiles
    KP = KS // 2  # DoubleRow pairs
    MT = b_shard // P
    nt = c_pad // 512

    nc = bacc.Bacc("TRN2", target_bir_lowering=False, debug=False)

    x_d = nc.dram_tensor("x", [b_shard, KS, P], FP8, kind="ExternalInput").ap()
    s_d = nc.dram_tensor("s", [P, KS, c_pad], FP8, kind="ExternalInput").ap()
    za_d = nc.dram_tensor("za", [P, c_pad], F32, kind="ExternalInput").ap()
    zb_d = nc.dram_tensor("zb", [P, c_pad], F32, kind="ExternalInput").ap()
    out_d = nc.dram_tensor("out", [b_shard, c], F32, kind="ExternalOutput").ap()

    with tile.TileContext(nc) as tc, ExitStack() as ctx:
        const = ctx.enter_context(tc.tile_pool(name="const", bufs=1))
        s_pool = ctx.enter_context(tc.tile_pool(name="s_res", bufs=1))
        x_pool = ctx.enter_context(tc.tile_pool(name="x", bufs=3))
        o_pool = ctx.enter_context(tc.tile_pool(name="o", bufs=3))
        ps_pool = ctx.enter_context(tc.tile_pool(name="ps", bufs=2 * nt, space="PSUM"))

        za = const.tile([P, c_pad], F32)
        zb = const.tile([P, c_pad], F32)
        nc.sync.dma_start(za[:], za_d[:])
        nc.sync.dma_start(zb[:], zb_d[:])

        # Resident S^T, loaded in ks-chunks so the first matmuls can start early.
        s_sb = s_pool.tile([P, KS, c_pad], FP8)
        for ck in range(0, KS, s_chunk):
            w = min(s_chunk, KS - ck)
            nc.sync.dma_start(s_sb[:, ck : ck + w, :], s_d[:, ck : ck + w, :])

        def body():
            for mt in range(MT):
                xt = x_pool.tile([P, KS, P], FP8, name="xt")
                nc.sync.dma_start(xt[:], x_d[ts(mt, P), :, :])

                psums = [ps_pool.tile([P, 512], F32, name=f"psum{i}") for i in range(nt)]
                for kp in range(KP):
                    w = xt[:, 2 * kp : 2 * kp + 2, :]
                    for ct in range(nt):
                        nc.tensor.matmul(
                            psums[ct][:],
                            w,
                            s_sb[:, 2 * kp : 2 * kp + 2, ts(ct, 512)],
                            start=(kp == 0),
                            stop=(kp == KP - 1),
                            perf_mode=mybir.MatmulPerfMode.DoubleRow,
                        )

                ot = o_pool.tile([P, c_pad], F32, name="ot")
                for ct in range(nt):
                    nc.vector.tensor_mul(ot[:, ts(ct, 512)], psums[ct][:], za[:, ts(ct, 512)])
                    nc.vector.tensor_add(ot[:, ts(ct, 512)], ot[:, ts(ct, 512)], zb[:, ts(ct, 512)])
                nc.sync.dma_start(out_d[ts(mt, P), :], ot[:, :c])

        if passes > 1 and loop:
            with tc.For_i(0, passes, 1):
                body()
        else:
            for _ in range(passes):
                body()

    nc.compile()
    _dedup_ldweights(nc)
    return nc


def build_nc_v2(b_shard=B_SHARD, k=K, c=C, c_pad=C_PAD, passes=1, loop=False,
                chunk_ks=16, m_group=4, trim=True):
    """Chunk-major variant: k-chunks outer, m-tiles inner (groups of m_group).

    The S^T load is consumed chunk-by-chunk right as it lands, so the 16MB
    prologue hides behind PE work instead of gating the first m-tile. x is
    DMAed in matching (m-tile, chunk) pieces. All m_group x nt psum tiles
    accumulate simultaneously (needs m_group * nt <= 8 banks).
    """
    KS = k // P
    MT = b_shard // P
    nt = c_pad // 512
    n_chunks = KS // chunk_ks
    kp_per_chunk = chunk_ks // 2
    n_groups = MT // m_group
    assert m_group * nt <= 8, "psum banks"
    # widths of the nt class tiles (last one trimmed to the real class count)
    widths = [512] * nt
    if trim:
        widths[-1] = c - 512 * (nt - 1)

    nc = bacc.Bacc("TRN2", target_bir_lowering=False, debug=False)

    x_d = nc.dram_tensor("x", [b_shard, KS, P], FP8, kind="ExternalInput").ap()
    s_d = nc.dram_tensor("s", [P, KS, c_pad], FP8, kind="ExternalInput").ap()
    za_d = nc.dram_tensor("za", [P, c_pad], F32, kind="ExternalInput").ap()
    zb_d = nc.dram_tensor("zb", [P, c_pad], F32, kind="ExternalInput").ap()
    out_d = nc.dram_tensor("out", [b_shard, c], F32, kind="ExternalOutput").ap()

    with tile.TileContext(nc) as tc, ExitStack() as ctx:
        const = ctx.enter_context(tc.tile_pool(name="const", bufs=1))
        s_pool = ctx.enter_context(tc.tile_pool(name="s_res", bufs=1))
        x_pool = ctx.enter_context(tc.tile_pool(name="x", bufs=3 * m_group))
        o_pool = ctx.enter_context(tc.tile_pool(name="o", bufs=3))
        ps_pool = ctx.enter_context(tc.tile_pool(name="ps", bufs=1, space="PSUM"))

        za = const.tile([P, c_pad], F32)
        zb = const.tile([P, c_pad], F32)
        nc.sync.dma_start(za[:], za_d[:])
        nc.sync.dma_start(zb[:], zb_d[:])

        s_sb = s_pool.tile([P, KS, c_pad], FP8)
        for ck in range(n_chunks):
            nc.sync.dma_start(
                s_sb[:, ts(ck, chunk_ks), :], s_d[:, ts(ck, chunk_ks), :]
            )

        def body():
            for g in range(n_groups):
                mts = range(g * m_group, (g + 1) * m_group)
                psums = {
                    (mt, ct): ps_pool.tile([P, 512], F32, name=f"ps{mt % m_group}_{ct}")
                    for mt in mts
                    for ct in range(nt)
                }
                for ck in range(n_chunks):
                    for mt in mts:
                        xc = x_pool.tile([P, chunk_ks, P], FP8, name="xc")
                        nc.sync.dma_start(
                            xc[:], x_d[ts(mt, P), ts(ck, chunk_ks), :]
                        )
                        for kpl in range(kp_per_chunk):
                            kp = ck * kp_per_chunk + kpl
                            w = xc[:, 2 * kpl : 2 * kpl + 2, :]
                            for ct in range(nt):
                                wd = widths[ct]
                                nc.tensor.matmul(
                                    psums[(mt, ct)][:, :wd],
                                    w,
                                    s_sb[:, 2 * kp : 2 * kp + 2,
                                         512 * ct : 512 * ct + wd],
                                    start=(kp == 0),
                                    stop=(kp == KP_TOT - 1),
                                    perf_mode=mybir.MatmulPerfMode.DoubleRow,
                                )
                for mt in mts:
                    ot = o_pool.tile([P, c_pad], F32, name="ot")
                    for ct in range(nt):
                        wd = widths[ct]
                        sl = slice(512 * ct, 512 * ct + wd)
                        nc.vector.tensor_mul(
                            ot[:, sl], psums[(mt, ct)][:, :wd], za[:, sl]
                        )
                        nc.vector.tensor_add(ot[:, sl], ot[:, sl], zb[:, sl])
                    nc.sync.dma_start(out_d[ts(mt, P), :], ot[:, :c])

        KP_TOT = KS // 2
        if passes > 1 and loop:
            with tc.For_i(0, passes, 1):
                body()
        else:
            for _ in range(passes):
                body()

    nc.compile()
    _dedup_ldweights(nc)
    return nc


def _dedup_ldweights(nc):
    """Drop back-to-back duplicate InstLdweights on the PE stream.

    Tile legalization splits every fp8 matmul into Ldweights+Matmult; the two
    class-tile matmuls of each (m-tile, k-pair) share identical weights, so
    the second load is redundant. Loaded PE weights persist across matmuls,
    and the duplicate carries no semaphore waits/updates, so removing it is
    invisible to scheduling. This halves the LDWEIGHTS stream, which is the
    PE bottleneck (DoubleRow matmuls run 2x faster than their weight loads).
    """
    import re

    pe = mybir.EngineType.PE
    for blk in nc.m.functions[0].blocks:
        insts = list(blk.instructions)
        keep, prev_sig, changed = [], None, False
        for i in insts:
            if i.engine == pe:
                tn = type(i).__name__
                if tn == "InstLdweights":
                    m = re.search(r"in=\[.*", i.concise())
                    sig = m.group(0) if m else None
                    if (
                        sig is not None
                        and sig == prev_sig
                        and not i.has_wait()
                        and not i.has_update()
                    ):
                        changed = True
                        continue  # drop duplicate
                    prev_sig = sig
                elif tn != "InstMatmult":
                    prev_sig = None  # other PE inst: invalidate
            keep.append(i)
        if changed:
            blk.instructions = keep


def _pack_x_shard(x_u8: np.ndarray, k: int) -> np.ndarray:
    """x_u8 [b, k] uint8 (0 / 0x38) -> [b, KS, 128] fp8 tiled K-major layout."""
    b = x_u8.shape[0]
    mt, ks = b // P, k // P
    t = x_u8.reshape(mt, P, ks, P)  # [mt, j, ks, p]
    t = np.ascontiguousarray(t.transpose(0, 3, 2, 1))  # [mt, p, ks, j]
    return t.reshape(b, ks, P).view(FP8_NP)


def preprocess(x_bits, S, z_mu, z_std, b_shard=B_SHARD, k=K, c=C, c_pad=C_PAD,
               n_cores=N_CORES):
    """Host-side: build per-core input maps."""
    x_u8 = np.ascontiguousarray(np.asarray(x_bits)).view(np.uint8) * np.uint8(ONE_FP8)

    S_u8 = np.zeros((c_pad, k), np.uint8)
    S_u8[:c] = np.ascontiguousarray(np.asarray(S)).view(np.uint8) * np.uint8(ONE_FP8)
    st = S_u8.T.reshape(k // P, P, c_pad)  # [ks, p, c]
    s_dev = np.ascontiguousarray(st.transpose(1, 0, 2)).view(FP8_NP)  # [p, ks, c]

    b_full = x_bits.shape[0]
    min_std = max(1e-6, 1.0 / (b_full**0.5))
    std_safe = np.maximum(np.asarray(z_std, np.float64), min_std)
    a = 1.0 / (std_safe * TEMPERATURE)
    bvec = -np.asarray(z_mu, np.float64) * a
    a_pad = np.zeros(c_pad, np.float32)
    b_pad = np.zeros(c_pad, np.float32)
    a_pad[:c] = a.astype(np.float32)
    b_pad[:c] = bvec.astype(np.float32)
    za = np.ascontiguousarray(np.broadcast_to(a_pad, (P, c_pad)))
    zb = np.ascontiguousarray(np.broadcast_to(b_pad, (P, c_pad)))

    in_maps = []
    for ci in range(n_cores):
        xs = x_u8[ci * b_shard : (ci + 1) * b_shard]
        in_maps.append(
            {"x": _pack_x_shard(xs, k), "s": s_dev, "za": za, "zb": zb}
        )
    return in_maps


_NC_CACHE = {}


def run(inputs: dict, trace: bool = False, builder=None, **kw):
    """Returns (full_output [B, C] f32, BassKernelResults)."""
    if builder is None:
        builder = build_nc_v2
    key = builder.__name__
    if key not in _NC_CACHE:
        _NC_CACHE[key] = builder()
    nc = _NC_CACHE[key]
    in_maps = preprocess(**inputs)
    res = run_bass_kernel_spmd(
        nc, in_maps, core_ids=list(range(N_CORES)), trace=trace, **kw
    )
    out = np.concatenate([r["out"] for r in res.results], axis=0)
    return out, res


def kernel(**inputs) -> np.ndarray:
    out, _ = run(inputs)
    return out
